# revision 1
# baseline (speedup 1.0000x reference)
"""Trainium2 Bass kernel for nn_Damping: MLP trunk -> huge output layer ->
tril scatter -> D = L @ L.T, distributed over 8 NeuronCores.

Strategy (tensor-parallel over the 131328-wide output layer):
  - Host: fold biases into augmented trunk weights; permute + pad Wo's columns
    into a "flipped column-major" layout so that the triangular scatter on
    device becomes a single dma_gather with 64-element-aligned windows.
  - Device (SPMD x8): trunk MLP replicated; each core streams its 1024x18432
    f32 Wo shard from HBM through PE matmuls (M=1, N=512, 8 K-chunks),
    producing an 18432-element slice of the permuted output vector o'.
    AllGather o' -> every core gathers the 512x512 matrix V = L'^T (L' = JLJ,
    J = flip) via dma_gather + masks, computes D' = V^T V with 16 matmuls,
    and writes D = J D' J via a flipped output DMA.

The math: L lower-triangular (diag = exp(o[:512]), strict-lower = o[512:] in
row-major tril order). With J the anti-identity, L' = J L J is upper
triangular and D = L L^T = J (L' L'^T) J.  Row k of V = L'^T is
  [ L[511, 511-k], L[510, 511-k], ..., L[512-k, 511-k], exp-diag(511-k), 0... ]
i.e. column (511-k) of L read bottom-up: its data starts at COLUMN 0, which is
what makes a fixed 512-wide gather window land the data in the right place.
"""

import sys

sys.path.insert(0, "/opt/trn_rl_repo")

import numpy as np

import concourse.bass as bass
import concourse.bacc as bacc
import concourse.mybir as mybir
import concourse.tile as tile
from concourse.ap import AP
from concourse import bass_utils

N = 512
HID = 1024
OUT = N + N * (N - 1) // 2  # 131328
NCORES = 8
KC = HID // 128  # 8 k-chunks of the 1024-dim contraction

F32 = mybir.dt.float32
I16 = mybir.dt.int16


def _seg_starts():
    """64-aligned start (in elements of o') of segment g, g=0..511.

    o'[0:512] holds the flipped diag; segment g (g>=1) holds the g
    strict-lower elements of L column (511-g), bottom-up, zero-padded to a
    multiple of 64 (the padding comes from zero columns of the permuted Wo).
    """
    starts = np.zeros(N, dtype=np.int64)
    pos = N
    for g in range(1, N):
        starts[g] = pos
        pos += 64 * ((g + 63) // 64)
    return starts, int(pos)


TSTART, OTOT = _seg_starts()  # OTOT == 147456
assert OTOT == 147456
OSH = OTOT // NCORES  # 18432 per-core o' shard
NT = OSH // 512  # 36 psum tiles per core
NWIN = OTOT // 64 - 8 + 1  # 2297 gather windows cover the buffer exactly


def _colmap():
    """colmap[t] = original Wo column (o element) feeding o'[t], or -1 (pad)."""
    cm = np.full(OTOT, -1, dtype=np.int64)
    t = np.arange(N)
    cm[0:N] = (N - 1) - t  # flipped diag: o'[t] = o[511-t]
    for g in range(1, N):
        i = np.arange(g)
        r = (N - 1) - i  # L row index, from 511 downward
        c = (N - 1) - g  # L col index
        cm[TSTART[g] + i] = N + r * (r - 1) // 2 + c
    return cm


COLMAP = _colmap()


def _gather_idx():
    """int16 [16, 32] wrapped index tile: window start / 64 per V row g."""
    idx = np.zeros(N, dtype=np.int64)
    idx[0] = N // 64  # row 0 has no off-diag data; any in-bounds window
    idx[1:] = TSTART[1:] // 64
    assert idx.max() < NWIN
    wrapped = np.zeros((16, N // 16), dtype=np.int16)
    for g in range(N):
        wrapped[g % 16, g // 16] = idx[g]
    # replicated across the 8 GPSIMD cores' 16-partition groups
    return np.tile(wrapped, (8, 1))


GIDX = _gather_idx()

_PROGRAM_CACHE = {}


def build_program(debug_taps=False, reps=1, stream_only=False, loop_n=1):
    key = ("nc", debug_taps, reps, stream_only, loop_n)
    if key in _PROGRAM_CACHE:
        return _PROGRAM_CACHE[key]

    nc = bacc.Bacc("TRN2", target_bir_lowering=False, debug=False,
                   num_devices=NCORES)

    x_d = nc.dram_tensor("x", [N], F32, kind="ExternalInput")
    w0_d = nc.dram_tensor("w0", [4 * 128 + 1, HID], F32, kind="ExternalInput")
    w1_d = nc.dram_tensor("w1", [KC * 128 + 1, HID], F32, kind="ExternalInput")
    w2_d = nc.dram_tensor("w2", [KC * 128 + 1, HID], F32, kind="ExternalInput")
    wo_d = nc.dram_tensor("wo", [HID, OSH], F32, kind="ExternalInput")
    wob_d = nc.dram_tensor("wob", [OSH], F32, kind="ExternalInput")
    gidx_d = nc.dram_tensor("gidx", list(GIDX.shape), I16, kind="ExternalInput")
    out_d = nc.dram_tensor("out", [N, N], F32, kind="ExternalOutput")
    if debug_taps:
        dbg_h2 = nc.dram_tensor("dbg_h2", [128, 9], F32, kind="ExternalOutput")
        dbg_of = nc.dram_tensor("dbg_ofull", [OTOT], F32, kind="ExternalOutput")
        dbg_lt = nc.dram_tensor("dbg_lt", [128, 4 * 512], F32,
                                kind="ExternalOutput")
        dbg_ltm = nc.dram_tensor("dbg_ltm", [128, 4 * 512], F32,
                                 kind="ExternalOutput")

    with tile.TileContext(nc) as tc:
        with (
            tc.tile_pool(name="wop", bufs=4) as wop,
            tc.tile_pool(name="trunkp", bufs=4) as trunkp,
            tc.tile_pool(name="persist", bufs=1) as persist,
            tc.tile_pool(name="stagep", bufs=2) as stagep,
            tc.tile_pool(name="psum", bufs=6, space="PSUM") as psum,
            tc.tile_pool(name="dram", bufs=2, space="DRAM") as dram,
        ):
          def _emit_body():
            # ---- static masks ------------------------------------------------
            # iota_t[p, j] = j - p; row-index of V chunk c at partition p is
            # g = 128c + p, so (j < g) <=> (iota < 128c), (j == g) <=> (== 128c)
            iota_t = persist.tile([128, 512], mybir.dt.int32, tag="iota")
            nc.gpsimd.iota(iota_t[:], pattern=[[1, 512]], base=0,
                           channel_multiplier=-1)
            ltm = []  # keep-mask: 1.0 where col < row-index (the off-diag data)
            eqm = []  # 1.0 where col == row-index (the diag position)
            for c in range(4):
                m = persist.tile([128, 512], F32, tag=f"ltm{c}")
                nc.vector.tensor_scalar(m[:], iota_t[:], 128 * c, None,
                                        mybir.AluOpType.is_lt)
                ltm.append(m)
                e = persist.tile([128, 512], F32, tag=f"eqm{c}")
                nc.vector.tensor_scalar(e[:], iota_t[:], 128 * c, None,
                                        mybir.AluOpType.is_equal)
                eqm.append(e)

            gidx_sb = persist.tile(list(GIDX.shape), I16, tag="gidx")
            nc.gpsimd.dma_start(gidx_sb[:], gidx_d[:])

            # ---- trunk: x -> h2 (all on partition-0 rows + kT transposes) ----
            def to_kT(src_ap_flat, n_elems, tag):
                """DRAM [n_elems] -> SBUF [128, n_elems//128 + 1] k-chunk
                layout with a trailing [1,0,..] column for the bias matmul."""
                ncols = n_elems // 128
                hk = persist.tile([128, ncols + 1], F32, tag=tag)
                nc.vector.memset(hk[:, ncols:ncols + 1], 0.0)
                nc.vector.memset(hk[0:1, ncols:ncols + 1], 1.0)
                # fine-strided (4B/partition) pattern: HWDGE wedges on it,
                # SWDGE (gpsimd) handles it
                nc.gpsimd.dma_start(
                    hk[:, 0:ncols],
                    AP(src_ap_flat.tensor, src_ap_flat.offset,
                       [[1, 128], [128, ncols]]),
                )
                return hk

            x_kT = to_kT(x_d.ap(), N, "xkT")

            def trunk_layer(h_kT, w_dram, kchunks, tag):
                wb = stagep.tile([1, HID], F32, tag="wbias")
                nc.sync.dma_start(wb[:], w_dram[kchunks * 128:kchunks * 128 + 1, :])
                h_sb = persist.tile([1, HID], F32, tag="h_sb")
                for nh in range(2):
                    ps = psum.tile([128, 512], F32, tag="ps")
                    for kc in range(kchunks):
                        wt = trunkp.tile([128, 512], F32, tag="wt")
                        nc.sync.dma_start(
                            wt[:],
                            w_dram[kc * 128:(kc + 1) * 128,
                                   nh * 512:(nh + 1) * 512],
                        )
                        nc.tensor.matmul(ps[0:1, :], h_kT[:, kc:kc + 1], wt[:],
                                         start=(kc == 0), stop=False)
                    nc.tensor.matmul(ps[0:1, :],
                                     h_kT[0:1, kchunks:kchunks + 1],
                                     wb[0:1, nh * 512:(nh + 1) * 512],
                                     start=False, stop=True)
                    nc.scalar.activation(h_sb[0:1, nh * 512:(nh + 1) * 512],
                                         ps[0:1, :],
                                         mybir.ActivationFunctionType.Tanh)
                # bounce through DRAM to re-layout [1,1024] -> [128, 8+1]
                hd = dram.tile([HID], F32, tag="hdram")
                nc.sync.dma_start(hd[:], h_sb[0:1, :])
                hdap = hd[:]
                return to_kT(hdap, HID, f"kT_{tag}")

            h0_kT = trunk_layer(x_kT, w0_d, 4, "l0")
            h1_kT = trunk_layer(h0_kT, w1_d, KC, "l1")
            h2_kT = trunk_layer(h1_kT, w2_d, KC, "l2")

            # ---- output layer: o'_shard = h2 @ Wo_shard + bo_shard -----------
            # Wo streamed as 4MB tiles (2 n-tiles of 512 each x 8 k-chunks),
            # alternating between the two HWDGE rings (sync / scalar) so the
            # per-DMA completion latency hides under the other ring's
            # transfer and the stream runs at HBM bandwidth.
            o_shard = dram.tile([OSH], F32, tag="oshard")
            for tt in range(NT // 2):
                eng = nc.sync if (tt % 2 == 0) else nc.scalar
                wt = wop.tile([128, 2 * KC * 512], F32, tag="wo")
                eng.dma_start(
                    wt[:],
                    AP(wo_d, tt * 1024,
                       [[OSH, 128], [128 * OSH, KC], [1, 1024]]),
                )
                for half in range(2):
                    t = 2 * tt + half
                    ps = psum.tile([128, 512], F32, tag="ps")
                    for kc in range(KC):
                        nc.tensor.matmul(
                            ps[0:1, :], h2_kT[:, kc:kc + 1],
                            wt[:, kc * 1024 + half * 512:
                               kc * 1024 + half * 512 + 512],
                            start=(kc == 0), stop=(kc == KC - 1))
                    wob_t = stagep.tile([1, 512], F32, tag="wob")
                    eng.dma_start(wob_t[:],
                                  AP(wob_d, t * 512, [[1, 1], [1, 512]]))
                    stage = stagep.tile([1, 512], F32, tag="stage")
                    nc.vector.tensor_add(stage[:], ps[0:1, :], wob_t[:])
                    o_ap = o_shard[:]
                    eng.dma_start(
                        AP(o_ap.tensor, o_ap.offset + t * 512,
                           [[1, 1], [1, 512]]),
                        stage[:],
                    )

            if stream_only:
                return
            # ---- AllGather the full o' ---------------------------------------
            o_full = dram.tile([OTOT], F32, tag="ofull")
            nc.gpsimd.collective_compute(
                "AllGather",
                mybir.AluOpType.bypass,
                ins=[o_shard[:].opt()],
                outs=[o_full[:].opt()],
                replica_groups=[list(range(NCORES))],
            )
            of_ap = o_full[:]

            # ---- diag: d[p, c] = exp(o'[128c + p]) ---------------------------
            d_raw = persist.tile([128, 4], F32, tag="draw")
            nc.gpsimd.dma_start(
                d_raw[:], AP(of_ap.tensor, of_ap.offset, [[1, 128], [128, 4]])
            )
            d_t = persist.tile([128, 4], F32, tag="dexp")
            nc.scalar.activation(d_t[:], d_raw[:],
                                 mybir.ActivationFunctionType.Exp)

            # ---- gather V = L'^T as [128, 4, 512] ----------------------------
            lt = persist.tile([128, 4, 512], F32, tag="lt")
            nc.gpsimd.dma_gather(
                lt[:],
                AP(of_ap.tensor, of_ap.offset, [[64, NWIN], [1, 512]]),
                gidx_sb[:],
                N,
                N,
                512,
                elem_step=64,
            )

            if debug_taps:
                nc.sync.dma_start(dbg_of[:], o_full[:])
                nc.sync.dma_start(dbg_lt[:], lt[:].rearrange("p a b -> p (a b)"))
                nc.sync.dma_start(dbg_h2[:], h2_kT[:])

            # ---- mask junk + insert exp-diag ---------------------------------
            tmp = persist.tile([128, 512], F32, tag="masktmp")
            for c in range(4):
                ltc = lt[:, c, :]
                nc.vector.tensor_mul(tmp[:], ltc, ltm[c][:])
                nc.vector.scalar_tensor_tensor(
                    ltc, eqm[c][:], d_t[:, c:c + 1], tmp[:],
                    mybir.AluOpType.mult, mybir.AluOpType.add,
                )
            if debug_taps:
                nc.sync.dma_start(dbg_ltm[:],
                                  lt[:].rearrange("p a b -> p (a b)"))

            # ---- D' = V^T V, written out flipped -----------------------------
            for m in range(4):
                psd = psum.tile([128, 512], F32, tag="ps")
                for c in range(4):
                    nc.tensor.matmul(psd[:], lt[:, c, m * 128:(m + 1) * 128],
                                     lt[:, c, :],
                                     start=(c == 0), stop=(c == 3))
                dout = stagep.tile([128, 512], F32, tag="dout")
                nc.vector.tensor_copy(dout[:], psd[:])
                # device emits D'; host flips both axes (D = J D' J)
                nc.sync.dma_start(
                    AP(out_d, 128 * m * N, [[N, 128], [1, 512]]),
                    dout[:],
                )

          if loop_n > 1:
            assert stream_only and reps == 1
            with tc.For_i(0, loop_n, 1):
                _emit_body()
          else:
            for _rep in range(reps):
                _emit_body()

    nc.compile()
    _PROGRAM_CACHE[key] = nc
    return nc


def prep_inputs(input, W0, b0, W1, b1, W2, b2, Wo, bo):
    """Host-side input prep: bias folding + Wo permutation/padding/sharding."""
    x = np.asarray(input, np.float32)
    w0a = np.concatenate([np.asarray(W0, np.float32),
                          np.asarray(b0, np.float32)[None, :]], axis=0)
    w1a = np.concatenate([np.asarray(W1, np.float32),
                          np.asarray(b1, np.float32)[None, :]], axis=0)
    w2a = np.concatenate([np.asarray(W2, np.float32),
                          np.asarray(b2, np.float32)[None, :]], axis=0)
    Wo = np.asarray(Wo, np.float32)
    bo = np.asarray(bo, np.float32)

    valid = COLMAP >= 0
    wo_perm = np.zeros((HID, OTOT), dtype=np.float32)
    wo_perm[:, valid] = Wo[:, COLMAP[valid]]
    wob_perm = np.zeros((OTOT,), dtype=np.float32)
    wob_perm[valid] = bo[COLMAP[valid]]

    in_maps = []
    for c in range(NCORES):
        sl = slice(c * OSH, (c + 1) * OSH)
        in_maps.append({
            "x": x,
            "w0": w0a,
            "w1": w1a,
            "w2": w2a,
            "wo": np.ascontiguousarray(wo_perm[:, sl]),
            "wob": np.ascontiguousarray(wob_perm[sl]),
            "gidx": GIDX,
        })
    return in_maps


def kernel(**inputs) -> np.ndarray:
    nc = build_program()
    in_maps = prep_inputs(**inputs)
    res = bass_utils.run_bass_kernel_spmd(nc, in_maps, list(range(NCORES)))
    dprime = res.results[0]["out"]
    return np.ascontiguousarray(dprime[::-1, ::-1]).reshape(1, N, N)


if __name__ == "__main__":
    # quick host-side check of the layout math against a numpy reference
    rng = np.random.default_rng(0)
    o = rng.standard_normal(OUT).astype(np.float32)
    # reference L
    L = np.zeros((N, N), np.float32)
    r, c = np.tril_indices(N, k=-1)
    L[r, c] = o[N:]
    L[np.arange(N), np.arange(N)] = np.exp(o[:N])
    D_ref = L @ L.T
    # o' = o[COLMAP] with zeros at padding
    op = np.zeros(OTOT, np.float32)
    op[COLMAP >= 0] = o[COLMAP[COLMAP >= 0]]
    # gather sim
    V = np.zeros((N, N), np.float32)
    idx = GIDX  # wrapped
    for g in range(N):
        w = int(idx[g % 16, g // 16]) * 64
        V[g, :] = op[w:w + 512]
    # masks
    col = np.arange(N)[None, :]
    row = np.arange(N)[:, None]
    V = V * (col < row)
    V = V + (col == row) * np.exp(op[:N])[:, None]
    Dp = V.T @ V
    D = Dp[::-1, ::-1]
    print("layout max err:", np.abs(D - D_ref).max(),
          "scale:", np.abs(D_ref).max())



# revision 9
# speedup vs baseline: 1.7724x; 1.7724x over previous
"""Trainium2 Bass kernel for nn_Damping: MLP trunk -> huge output layer ->
tril scatter -> D = L @ L.T, distributed over 8 NeuronCores.

Strategy (tensor-parallel over the 131328-wide output layer), v2:
  - Host: fold biases into augmented trunk weights; permute + pad Wo's columns
    into a "flipped column-major" layout so that the triangular scatter on
    device becomes dma_gathers with 64-element-aligned windows. Cast trunk +
    Wo weights to bf16 (PSUM accumulates fp32; 2e-2 tolerance, ~1e-3 actual).
  - Device (SPMD x8): trunk MLP replicated in bf16; each core streams its
    1024x18944 bf16 Wo shard from HBM through PE matmuls (M=1). The first
    512 columns are the (redundant, replicated) flipped-diag head so exp()
    of the diagonal runs ~20us into the stream with no cross-core dep.
  - The remaining 36 o'-tiles are laid out AllGather-chunk-interleaved:
    o_full chunk k = concat over cores of their k-th 4608-elem sub-shard.
    After each 9-tile chunk completes, its AllGather fires on the CC stream
    (overlapped with the continuing Wo stream), then a 128-row dma_gather of
    V-rows, masking, and 4 accumulating V^T V matmuls pipeline behind it --
    the quadratic tril layout guarantees row-group k's gather windows only
    touch o_full chunks <= k.
  - D' accumulates in 4 persistent PSUM banks; final copy + flipped output
    DMA (host un-flips: D = J D' J).

The math: L lower-triangular (diag = exp(o[:512]), strict-lower = o[512:] in
row-major tril order). With J the anti-identity, L' = J L J is upper
triangular and D = L L^T = J (L' L'^T) J.  Row k of V = L'^T is
  [ L[511, 511-k], L[510, 511-k], ..., L[512-k, 511-k], exp-diag(511-k), 0... ]
i.e. column (511-k) of L read bottom-up: its data starts at COLUMN 0, which is
what makes a fixed 512-wide gather window land the data in the right place.
"""

import sys

sys.path.insert(0, "/opt/trn_rl_repo")

import numpy as np

import concourse.bass as bass
import concourse.bacc as bacc
import concourse.mybir as mybir
import concourse.tile as tile
from concourse.ap import AP
from concourse import bass_utils

N = 512
HID = 1024
OUT = N + N * (N - 1) // 2  # 131328
NCORES = 8
KC = HID // 128  # 8 k-chunks of the 1024-dim contraction

F32 = mybir.dt.float32
BF16 = mybir.dt.bfloat16
I16 = mybir.dt.int16


def _seg_starts():
    """64-aligned start (in elements of o') of segment g, g=0..511.

    o'[0:512] holds the flipped diag; segment g (g>=1) holds the g
    strict-lower elements of L column (511-g), bottom-up, zero-padded to a
    multiple of 64 (the padding comes from zero columns of the permuted Wo).
    """
    starts = np.zeros(N, dtype=np.int64)
    pos = N
    for g in range(1, N):
        starts[g] = pos
        pos += 64 * ((g + 63) // 64)
    return starts, int(pos)


TSTART, OTOT = _seg_starts()  # OTOT == 147456
assert OTOT == 147456
OSH = OTOT // NCORES  # 18432 per-core o' shard (excl. the diag head tile)
NTS = OSH // 512  # 36 shard tiles per core
NCHUNK = 4
TPC = NTS // NCHUNK  # 9 tiles per AllGather chunk
CE = TPC * 512  # 4608 per-core chunk elems
OCE = CE * NCORES  # 36864 o_full elems per chunk
PERCORE = 512 + OSH  # 18944 Wo columns per core (diag head + shard)
NPAIR = NTS // 2  # 18 paired-tile DMA groups
PREFETCH = 6  # pair DMAs issued before the scalar engine blocks on exp()

# V row-group k (rows 128k..128k+127) gather windows must lie inside
# o_full chunks 0..k:
for _k in range(NCHUNK):
    _gmax = 128 * (_k + 1) - 1
    assert int(TSTART[_gmax]) + 512 <= OCE * (_k + 1), (_k, TSTART[_gmax])


def _colmap():
    """colmap[t] = original Wo column (o element) feeding o'[t], or -1 (pad)."""
    cm = np.full(OTOT, -1, dtype=np.int64)
    t = np.arange(N)
    cm[0:N] = (N - 1) - t  # flipped diag: o'[t] = o[511-t]
    for g in range(1, N):
        i = np.arange(g)
        r = (N - 1) - i  # L row index, from 511 downward
        c = (N - 1) - g  # L col index
        cm[TSTART[g] + i] = N + r * (r - 1) // 2 + c
    return cm


COLMAP = _colmap()


def _gather_idx():
    """int16 [128, 32] wrapped index tile: window start / 64 per V row g."""
    idx = np.zeros(N, dtype=np.int64)
    idx[0] = N // 64  # row 0 has no off-diag data; any in-bounds window
    idx[1:] = TSTART[1:] // 64
    wrapped = np.zeros((16, N // 16), dtype=np.int16)
    for g in range(N):
        wrapped[g % 16, g // 16] = idx[g]
    # replicated across the 8 GPSIMD cores' 16-partition groups
    return np.tile(wrapped, (8, 1))


GIDX = _gather_idx()

_PROGRAM_CACHE = {}


def build_program():
    key = "nc"
    if key in _PROGRAM_CACHE:
        return _PROGRAM_CACHE[key]

    nc = bacc.Bacc("TRN2", target_bir_lowering=False, debug=False,
                   num_devices=NCORES)

    x_d = nc.dram_tensor("x", [N], BF16, kind="ExternalInput")
    w0_d = nc.dram_tensor("w0", [4 * 128 + 1, HID], BF16, kind="ExternalInput")
    w1_d = nc.dram_tensor("w1", [KC * 128 + 1, HID], BF16, kind="ExternalInput")
    w2_d = nc.dram_tensor("w2", [KC * 128 + 1, HID], BF16, kind="ExternalInput")
    wo_d = nc.dram_tensor("wo", [HID, PERCORE], BF16, kind="ExternalInput")
    wob_d = nc.dram_tensor("wob", [PERCORE], F32, kind="ExternalInput")
    gidx_d = nc.dram_tensor("gidx", list(GIDX.shape), I16, kind="ExternalInput")
    out_d = nc.dram_tensor("out", [N, N], F32, kind="ExternalOutput")

    with tile.TileContext(nc) as tc:
        with (
            tc.tile_pool(name="wop", bufs=7) as wop,
            tc.tile_pool(name="trunkp", bufs=4) as trunkp,
            tc.tile_pool(name="persist", bufs=1) as persist,
            tc.tile_pool(name="stagep", bufs=4) as stagep,
            tc.tile_pool(name="psum", bufs=4, space="PSUM") as psum,
            tc.tile_pool(name="psumd", bufs=1, space="PSUM") as psumd,
            tc.tile_pool(name="dram", bufs=2, space="DRAM") as dram,
        ):
            # ---- static masks ------------------------------------------------
            # iota_t[p, j] = j - p; row-index of V chunk c at partition p is
            # g = 128c + p, so (j < g) <=> (iota < 128c), (j == g) <=> (== 128c)
            iota_t = persist.tile([128, 512], mybir.dt.int32, tag="iota")
            nc.gpsimd.iota(iota_t[:], pattern=[[1, 512]], base=0,
                           channel_multiplier=-1)
            ltm = []  # keep-mask: 1.0 where col < row-index (the off-diag data)
            eqm = []  # 1.0 where col == row-index (the diag position)
            for c in range(NCHUNK):
                m = persist.tile([128, 512], F32, tag=f"ltm{c}")
                nc.vector.tensor_scalar(m[:], iota_t[:], 128 * c, None,
                                        mybir.AluOpType.is_lt)
                ltm.append(m)
                e = persist.tile([128, 512], F32, tag=f"eqm{c}")
                nc.vector.tensor_scalar(e[:], iota_t[:], 128 * c, None,
                                        mybir.AluOpType.is_equal)
                eqm.append(e)

            gidx_sb = persist.tile(list(GIDX.shape), I16, tag="gidx")
            nc.gpsimd.dma_start(gidx_sb[:], gidx_d[:])

            # ---- trunk: x -> h2 (all on partition-0 rows + kT transposes) ----
            def to_kT(src_ap_flat, n_elems, tag):
                """DRAM [n_elems] -> SBUF [128, n_elems//128 + 1] k-chunk
                layout with a trailing [1,0,..] column for the bias matmul."""
                ncols = n_elems // 128
                hk = persist.tile([128, ncols + 1], BF16, tag=tag)
                nc.vector.memset(hk[:, ncols:ncols + 1], 0.0)
                nc.vector.memset(hk[0:1, ncols:ncols + 1], 1.0)
                # fine-strided (2B/partition) pattern: HWDGE wedges on it,
                # SWDGE (gpsimd) handles it
                nc.gpsimd.dma_start(
                    hk[:, 0:ncols],
                    AP(src_ap_flat.tensor, src_ap_flat.offset,
                       [[1, 128], [128, ncols]]),
                )
                return hk

            x_kT = to_kT(x_d.ap(), N, "xkT")

            def trunk_layer(h_kT, w_dram, kchunks, tag):
                wb = stagep.tile([1, HID], BF16, tag="wbias")
                nc.sync.dma_start(wb[:],
                                  w_dram[kchunks * 128:kchunks * 128 + 1, :])
                h_sb = persist.tile([1, HID], BF16, tag=f"h_sb_{tag}")
                for nh in range(2):
                    ps = psum.tile([128, 512], F32, tag="ps")
                    for kcc in range(kchunks):
                        wt = trunkp.tile([128, 512], BF16, tag="wt")
                        nc.sync.dma_start(
                            wt[:],
                            w_dram[kcc * 128:(kcc + 1) * 128,
                                   nh * 512:(nh + 1) * 512],
                        )
                        nc.tensor.matmul(ps[0:1, :], h_kT[:, kcc:kcc + 1],
                                         wt[:], start=(kcc == 0), stop=False)
                    nc.tensor.matmul(ps[0:1, :],
                                     h_kT[0:1, kchunks:kchunks + 1],
                                     wb[0:1, nh * 512:(nh + 1) * 512],
                                     start=False, stop=True)
                    nc.scalar.activation(h_sb[0:1, nh * 512:(nh + 1) * 512],
                                         ps[0:1, :],
                                         mybir.ActivationFunctionType.Tanh)
                # bounce through DRAM to re-layout [1,1024] -> [128, 8+1]
                hd = dram.tile([HID], BF16, tag="hdram")
                nc.sync.dma_start(hd[:], h_sb[0:1, :])
                return to_kT(hd[:], HID, f"kT_{tag}")

            h0_kT = trunk_layer(x_kT, w0_d, 4, "l0")
            h1_kT = trunk_layer(h0_kT, w1_d, KC, "l1")
            h2_kT = trunk_layer(h1_kT, w2_d, KC, "l2")

            # ---- persistent state for the pipelined tail ---------------------
            lt = persist.tile([128, NCHUNK, 512], F32, tag="lt")
            ltb = persist.tile([128, NCHUNK, 512], BF16, tag="ltb")
            tmp = persist.tile([128, 512], F32, tag="masktmp")
            d_raw = persist.tile([128, 4], F32, tag="draw")
            d_t = persist.tile([128, 4], F32, tag="dexp")
            psd = [psumd.tile([128, 512], F32, tag=f"psd{m}",
                              name=f"psd{m}") for m in range(4)]

            dscr = dram.tile([N], F32, tag="dscr")
            o_shard = dram.tile([OSH], F32, tag="oshard")
            o_full = dram.tile([OTOT], F32, tag="ofull")
            of_ap = o_full[:]
            os_ap = o_shard[:]

            # ---- output-layer tile consumer ----------------------------------
            def do_tile(T, wt_ap, eng):
                """o' tile T (T=0: diag head -> dscr; T>=1: shard tile)."""
                ps = psum.tile([128, 512], F32, tag="ps")
                for kcc in range(KC):
                    nc.tensor.matmul(
                        ps[0:1, :], h2_kT[:, kcc:kcc + 1],
                        wt_ap(kcc),
                        start=(kcc == 0), stop=(kcc == KC - 1))
                wob_t = stagep.tile([1, 512], F32, tag="wob")
                eng.dma_start(wob_t[:],
                              AP(wob_d, T * 512, [[1, 1], [1, 512]]))
                stage = stagep.tile([1, 512], F32, tag="stage")
                nc.vector.tensor_add(stage[:], ps[0:1, :], wob_t[:])
                if T == 0:
                    dst = dscr[:]
                    eng.dma_start(
                        AP(dst.tensor, dst.offset, [[1, 1], [1, 512]]),
                        stage[:])
                else:
                    eng.dma_start(
                        AP(os_ap.tensor, os_ap.offset + (T - 1) * 512,
                           [[1, 1], [1, 512]]),
                        stage[:])

            # ---- chunk tail, stage 1: AllGather -> V-row gather (gpsimd) -----
            def chunk_ag(k):
                nc.gpsimd.collective_compute(
                    "AllGather",
                    mybir.AluOpType.bypass,
                    ins=[o_shard[k * CE:(k + 1) * CE].opt()],
                    outs=[o_full[k * OCE:(k + 1) * OCE].opt()],
                    replica_groups=[list(range(NCORES))],
                )
                nwin = (OCE * (k + 1)) // 64 - 8 + 1
                nc.gpsimd.dma_gather(
                    lt[:, k:k + 1, :],
                    AP(of_ap.tensor, of_ap.offset, [[64, nwin], [1, 512]]),
                    gidx_sb[:, 8 * k:8 * (k + 1)],
                    128,
                    128,
                    512,
                    elem_step=64,
                )

            # ---- chunk tail, stage 2 (emitted ~1 chunk later so the DVE
            # never stalls mid-stream): mask + diag insert + V^T V accum -----
            def chunk_masks(k):
                nc.vector.tensor_mul(tmp[:], lt[:, k, :], ltm[k][:])
                nc.vector.scalar_tensor_tensor(
                    ltb[:, k, :], eqm[k][:], d_t[:, k:k + 1], tmp[:],
                    mybir.AluOpType.mult, mybir.AluOpType.add,
                )
                for m in range(4):
                    nc.tensor.matmul(
                        psd[m][:], ltb[:, k, m * 128:(m + 1) * 128],
                        ltb[:, k, :],
                        start=(k == 0), stop=(k == NCHUNK - 1),
                        skip_group_check=True)

            # ---- diag head tile (replicated): d = exp(o'_diag) early ---------
            wtd = wop.tile([128, KC * 512], BF16, tag="wod", bufs=1)
            nc.sync.dma_start(
                wtd[:],
                AP(wo_d, 0, [[PERCORE, 128], [128 * PERCORE, KC], [1, 512]]),
            )
            do_tile(0, lambda kcc: wtd[:, kcc * 512:(kcc + 1) * 512],
                    nc.scalar)

            # ---- software-pipelined Wo stream --------------------------------
            pair_tiles = {}

            def issue(p):
                eng = nc.sync if (p % 2 == 0) else nc.scalar
                wt = wop.tile([128, 2 * KC * 512], BF16, tag="wo")
                eng.dma_start(
                    wt[:],
                    AP(wo_d, 512 + p * 1024,
                       [[PERCORE, 128], [128 * PERCORE, KC], [1, 1024]]),
                )
                pair_tiles[p] = wt

            for p in range(PREFETCH):
                issue(p)

            # exp(diag): scalar blocks here ~15us with ring-B DMAs queued
            nc.gpsimd.dma_start(
                d_raw[:],
                AP(dscr[:].tensor, dscr[:].offset, [[1, 128], [128, 4]]))
            nc.scalar.activation(d_t[:], d_raw[:],
                                 mybir.ActivationFunctionType.Exp)

            for p in range(NPAIR):
                eng = nc.sync if (p % 2 == 0) else nc.scalar
                wt = pair_tiles.pop(p)
                for half in range(2):
                    s = 2 * p + half  # shard tile index 0..35
                    do_tile(
                        1 + s,
                        lambda kcc, h=half: wt[:, kcc * 1024 + h * 512:
                                               kcc * 1024 + h * 512 + 512],
                        eng)
                    if s % TPC == TPC - 1:
                        k = s // TPC
                        if k >= 1:
                            # masks for the previous chunk: its gather landed
                            # ~12us ago, so the DVE ops don't stall
                            chunk_masks(k - 1)
                        chunk_ag(k)
                if p + PREFETCH < NPAIR:
                    issue(p + PREFETCH)
            chunk_masks(NCHUNK - 1)

            # ---- D' out (host flips both axes: D = J D' J) -------------------
            for m in range(4):
                dout = stagep.tile([128, 512], F32, tag="dout")
                nc.vector.tensor_copy(dout[:], psd[m][:])
                nc.sync.dma_start(
                    AP(out_d, 128 * m * N, [[N, 128], [1, 512]]),
                    dout[:],
                )

    nc.compile()
    _PROGRAM_CACHE[key] = nc
    return nc


def prep_inputs(input, W0, b0, W1, b1, W2, b2, Wo, bo):
    """Host-side input prep: bias folding + Wo permutation/padding/sharding."""
    import ml_dtypes
    BF = ml_dtypes.bfloat16

    x = np.asarray(input, np.float32).astype(BF)
    w0a = np.concatenate([np.asarray(W0, np.float32),
                          np.asarray(b0, np.float32)[None, :]],
                         axis=0).astype(BF)
    w1a = np.concatenate([np.asarray(W1, np.float32),
                          np.asarray(b1, np.float32)[None, :]],
                         axis=0).astype(BF)
    w2a = np.concatenate([np.asarray(W2, np.float32),
                          np.asarray(b2, np.float32)[None, :]],
                         axis=0).astype(BF)
    Wo = np.asarray(Wo, np.float32)
    bo = np.asarray(bo, np.float32)

    valid = COLMAP >= 0
    wo_perm = np.zeros((HID, OTOT), dtype=BF)
    wo_perm[:, valid] = Wo[:, COLMAP[valid]].astype(BF)
    wob_perm = np.zeros((OTOT,), dtype=np.float32)
    wob_perm[valid] = bo[COLMAP[valid]]

    diag_w = wo_perm[:, 0:N]  # flipped-diag head, replicated on every core
    diag_b = wob_perm[0:N]
    # AllGather-chunk interleave: core c's shard = concat over chunks k of
    # logical o' positions [k*OCE + c*CE, k*OCE + (c+1)*CE)
    wo_resh = wo_perm.reshape(HID, NCHUNK, NCORES, CE)
    wob_resh = wob_perm.reshape(NCHUNK, NCORES, CE)

    in_maps = []
    for c in range(NCORES):
        wo_core = np.concatenate(
            [diag_w, wo_resh[:, :, c, :].reshape(HID, OSH)], axis=1)
        wob_core = np.concatenate(
            [diag_b, wob_resh[:, c, :].reshape(OSH)])
        in_maps.append({
            "x": x,
            "w0": w0a,
            "w1": w1a,
            "w2": w2a,
            "wo": np.ascontiguousarray(wo_core),
            "wob": np.ascontiguousarray(wob_core),
            "gidx": GIDX,
        })
    return in_maps


def kernel(**inputs) -> np.ndarray:
    nc = build_program()
    in_maps = prep_inputs(**inputs)
    res = bass_utils.run_bass_kernel_spmd(nc, in_maps, list(range(NCORES)))
    dprime = res.results[0]["out"]
    return np.ascontiguousarray(dprime[::-1, ::-1]).reshape(1, N, N)


if __name__ == "__main__":
    # quick host-side check of the layout math against a numpy reference
    rng = np.random.default_rng(0)
    o = rng.standard_normal(OUT).astype(np.float32)
    # reference L
    L = np.zeros((N, N), np.float32)
    r, c = np.tril_indices(N, k=-1)
    L[r, c] = o[N:]
    L[np.arange(N), np.arange(N)] = np.exp(o[:N])
    D_ref = L @ L.T
    # o' = o[COLMAP] with zeros at padding
    op = np.zeros(OTOT, np.float32)
    op[COLMAP >= 0] = o[COLMAP[COLMAP >= 0]]
    # gather sim (chunked)
    V = np.zeros((N, N), np.float32)
    for g in range(N):
        w = int(GIDX[g % 16, g // 16]) * 64
        k = g // 128
        assert w * 1 + 512 <= OCE * (k + 1) or g == 0
        V[g, :] = op[w:w + 512]
    col = np.arange(N)[None, :]
    row = np.arange(N)[:, None]
    V = V * (col < row)
    V = V + (col == row) * np.exp(op[:N])[:, None]
    Dp = V.T @ V
    D = Dp[::-1, ::-1]
    print("layout max err:", np.abs(D - D_ref).max(),
          "scale:", np.abs(D_ref).max())
    # chunk-interleave round-trip: rebuild logical o' from per-core shards
    sh = np.arange(OTOT).reshape(NCHUNK, NCORES, CE)
    rebuilt = np.zeros(OTOT, np.int64)
    for cc in range(NCORES):
        core_slice = sh[:, cc, :].reshape(OSH)  # shard tile order
        for k in range(NCHUNK):
            rebuilt[k * OCE + cc * CE:(k + 1 - 1) * OCE + cc * CE + CE] = \
                core_slice[k * CE:(k + 1) * CE]
    assert (rebuilt == np.arange(OTOT)).all()
    print("chunk interleave OK")
